# revision 35
# baseline (speedup 1.0000x reference)
"""Trainium2 Bass kernel for nn_MHAttentionLayer_64587718197528.

Reference computation (B=4, L=1024, D_MODEL=1024, S=2048, T=NUM_TOKENS=1000,
H=16, E=256, D_LLM=4096):
    q = (X @ Wq.T + bq)            [B*L, H*E]      X = target_embedding
    k = (SE @ Wk.T + bk)           [S, H*E]        SE = source_embedding
    v = (VE @ Wv.T + bv)           [S, H*E]        VE = value_embedding
    scores[b,h,l,s] = q . k / 16 ; A = softmax_s ; out = A @ v
    y = out @ Wo.T + bo            [B*L, D_LLM]

Sharding: tensor-parallel over heads. Core i owns heads {2i, 2i+1} (an
e-slice of 512 of the H*E dim). Each core computes its q/k/v projections,
attention for its 2 heads, and a partial out-projection
  partial_i = attn_out_i @ Wo[:, sl_i].T          [B*L, D_LLM]
The host sums the 8 partials and adds bo (linearity of the projection).

All matmul operands are float32r (full PE rate, ~1e-4 rel err). Phases:
  KV:   kT[512,2048] = Wk_i @ SE.T and v[2048,512] = VE_aug @ Wv_aug
        (bias for v folded via ones-row augmentation), SBUF-resident.
  Attn: per l-chunk of 512: q-projection (into SBUF), scoresT[s,l] in PSUM,
        exp on ACT (scale=1/16; no max subtraction -- |scaled scores| < ~8),
        softmax denominators via DVE accumulation + one ones-column matmul,
        outT[e,l] = v.T @ A.T accumulated on PE, normalized by 1/sums
        broadcast (outer-product matmul) on DVE.
  Proj: partial = outT.T @ Wo_i.T per [128,512] tile, DVE-evicted to DRAM.
"""
import numpy as np

# ---- problem constants (hardcoded per contract) ----
B, L, D = 4, 1024, 1024
S, T = 2048, 1000
H, E = 16, 256
DL = 4096
BL = B * L            # 4096 query rows
EC = 512              # e-slice per core (2 heads)
NCORES = 8
T1 = T + 1            # augmented contraction for v bias

_CACHE = {}
MM_DTYPE = "bf16"     # "f32r" (safe, ~1e-4) or "bf16" (2x PE rate, ~1e-3)


def _build_nc():
    from contextlib import ExitStack

    import concourse.tile as tile
    from concourse import bacc, mybir

    F32 = mybir.dt.float32
    F32R = mybir.dt.float32r
    MMD = mybir.dt.bfloat16 if MM_DTYPE == "bf16" else F32R
    AF = mybir.ActivationFunctionType
    MUL = mybir.AluOpType.mult
    ADD = mybir.AluOpType.add

    nc = bacc.Bacc("TRN2", target_bir_lowering=False, debug=False,
                   num_devices=NCORES)

    xt = nc.dram_tensor("xt", [D, BL], MMD, kind="ExternalInput")
    st = nc.dram_tensor("st", [T, S], MMD, kind="ExternalInput")
    vt = nc.dram_tensor("vt", [T1, S], MMD, kind="ExternalInput")
    wqt = nc.dram_tensor("wqt", [D, EC], MMD, kind="ExternalInput")
    wkt = nc.dram_tensor("wkt", [T, EC], MMD, kind="ExternalInput")
    wvt = nc.dram_tensor("wvt", [T1, EC], MMD, kind="ExternalInput")
    wot = nc.dram_tensor("wot", [EC, DL], MMD, kind="ExternalInput")
    bq_d = nc.dram_tensor("bq", [EC], F32, kind="ExternalInput")
    bk_d = nc.dram_tensor("bk", [EC], F32, kind="ExternalInput")
    out_d = nc.dram_tensor("out", [BL, DL], F32, kind="ExternalOutput")

    NKD = 8            # k-tiles for D=1024
    NKT = 8            # k-tiles for T=1000/T1=1001 (last partial)
    NLC = BL // 512    # 8 l-chunks
    NLT = BL // 128    # 32 l-tiles
    NST = S // 128     # 16 s-tiles
    NDC = DL // 512    # 8 out-proj chunks

    def kp_of(kk, total):
        return min(128, total - kk * 128)

    with tile.TileContext(nc) as tc:
        with ExitStack() as root:
            root.enter_context(
                nc.allow_low_precision(reason="fp32r matmul pipeline"))

            # ---- persistent pools ----
            consts = root.enter_context(tc.tile_pool(name="consts", bufs=1))
            kvp = root.enter_context(tc.tile_pool(name="kv", bufs=1))
            outp = root.enter_context(tc.tile_pool(name="outT", bufs=1))

            ones_f = consts.tile([128, 128], F32, name="ones_f")
            nc.vector.memset(ones_f[:], 1.0)
            ones_m = consts.tile([128, 128], F32R, name="ones_m")
            nc.vector.tensor_copy(ones_m[:], ones_f[:])
            ones_col = ones_m[:, 0:1]     # [128, 1] fp32r
            ones_row = ones_m[0:1, :]     # [1, 128] fp32r
            bqbk_t = consts.tile([128, 8], F32, name="bqbk_t")
            nc.sync.dma_start(bqbk_t[:, 0:4],
                              bq_d.ap().rearrange("(m p) -> p m", p=128))
            nc.sync.dma_start(bqbk_t[:, 4:8],
                              bk_d.ap().rearrange("(m p) -> p m", p=128))
            bq_t = bqbk_t[:, 0:4]
            bk_t = bqbk_t[:, 4:8]

            # kT: 4 e-tiles x [128, S]; v: 4 tiles [128, 4*EC] (4 s-tiles each)
            kt_sb = [kvp.tile([128, S], MMD, name=f"kt{m}", tag=f"kt{m}")
                     for m in range(4)]
            v_sb = [kvp.tile([128, 4 * EC], MMD, name=f"v{g}", tag=f"v{g}")
                    for g in range(4)]
            # outT: 4 e-tiles x [128, BL]
            o_sb = [outp.tile([128, BL], MMD, name=f"oT{m}", tag=f"oT{m}")
                    for m in range(4)]

            # wq lives in a root pool; loads are emitted after the KV
            # weight loads so they prefetch during KV compute
            wq_pool = root.enter_context(tc.tile_pool(name="wq", bufs=1))
            wq_t = []

            # ---- phase KV: kT = Wk_i @ SE.T ; v = VE_aug @ Wv_aug ----
            with ExitStack() as ph:
                ph.enter_context(nc.named_scope("kvproj"))
                wk_pool = ph.enter_context(tc.tile_pool(name="wk", bufs=1))
                wv_pool = ph.enter_context(tc.tile_pool(name="wv", bufs=1))
                sk_pool = ph.enter_context(tc.tile_pool(name="sk", bufs=4))
                sv_pool = ph.enter_context(tc.tile_pool(name="sv", bufs=4))
                psk = ph.enter_context(
                    tc.tile_pool(name="psk", bufs=1, space="PSUM"))
                psv = ph.enter_context(
                    tc.tile_pool(name="psv", bufs=1, space="PSUM"))
                wk_t, wv_t = [], []
                for kk in range(NKT):
                    kp = kp_of(kk, T)
                    w = wk_pool.tile([128, EC], MMD, name=f"wk{kk}")
                    nc.sync.dma_start(w[:kp, :], wkt[kk * 128:kk * 128 + kp, :])
                    wk_t.append(w)
                    kp1 = kp_of(kk, T1)
                    w2 = wv_pool.tile([128, EC], MMD, name=f"wv{kk}")
                    nc.sync.dma_start(w2[:kp1, :],
                                      wvt[kk * 128:kk * 128 + kp1, :])
                    wv_t.append(w2)
                for kk in range(NKD):
                    w = wq_pool.tile([128, EC], MMD, name=f"wq{kk}")
                    nc.sync.dma_start(w[:], wqt[kk * 128:(kk + 1) * 128, :])
                    wq_t.append(w)
                for sc in range(S // 512):
                    ps_k = [psk.tile([128, 512], F32, tag=f"psk{m}",
                                     name=f"psk{m}") for m in range(4)]
                    for kk in range(NKT):
                        kp = kp_of(kk, T)
                        stt = sk_pool.tile([128, 512], MMD, tag="stt",
                                           name="stt")
                        nc.sync.dma_start(
                            stt[:kp, :], st[kk * 128:kk * 128 + kp,
                                            sc * 512:(sc + 1) * 512])
                        for m in range(4):
                            nc.tensor.matmul(
                                ps_k[m][:], wk_t[kk][:kp, m * 128:(m + 1) * 128],
                                stt[:kp, :], start=(kk == 0),
                                stop=(kk == NKT - 1))
                    for m in range(4):
                        nc.scalar.activation(
                            kt_sb[m][:, sc * 512:(sc + 1) * 512], ps_k[m][:],
                            AF.Identity, bias=bk_t[:, m:m + 1])
                    # v group for the same 512-wide s range
                    g = sc
                    ps_v = [psv.tile([128, 512], F32, tag=f"psv{j}",
                                     name=f"psv{j}") for j in range(4)]
                    for kk in range(NKT):
                        kp1 = kp_of(kk, T1)
                        vtt = sv_pool.tile([128, 512], MMD, tag="vtt",
                                           name="vtt")
                        nc.sync.dma_start(
                            vtt[:kp1, :], vt[kk * 128:kk * 128 + kp1,
                                             g * 512:(g + 1) * 512])
                        for j in range(4):
                            nc.tensor.matmul(
                                ps_v[j][:], vtt[:kp1, j * 128:(j + 1) * 128],
                                wv_t[kk][:kp1, :], start=(kk == 0),
                                stop=(kk == NKT - 1))
                    for j in range(4):
                        nc.scalar.activation(
                            v_sb[g][:, j * EC:(j + 1) * EC], ps_v[j][:],
                            AF.Copy)

            # ---- fused attention phase (q-projection + attention per lc) ----
            with ExitStack() as ph:
                ph.enter_context(nc.named_scope("attn"))
                xq_pool = ph.enter_context(tc.tile_pool(name="xq", bufs=4))
                qt_pool = ph.enter_context(tc.tile_pool(name="qtp", bufs=2))
                a_pool = ph.enter_context(tc.tile_pool(name="ap", bufs=2))
                acc_pool = ph.enter_context(tc.tile_pool(name="accp", bufs=1))
                bc_pool = ph.enter_context(tc.tile_pool(name="bcp", bufs=2))
                psq_p = ph.enter_context(
                    tc.tile_pool(name="psq", bufs=1, space="PSUM"))
                ps_sT_p = ph.enter_context(
                    tc.tile_pool(name="ps_sT", bufs=1, space="PSUM"))
                ps_b_p = ph.enter_context(
                    tc.tile_pool(name="ps_b", bufs=2, space="PSUM"))
                ps_o_p = ph.enter_context(
                    tc.tile_pool(name="ps_o", bufs=2, space="PSUM"))

                def qproj_half(lc, half, qt_t):
                    ps_q = [psq_p.tile([128, 512], F32, tag=f"psq{mh}",
                                       name=f"psq{mh}") for mh in range(2)]
                    for kk in range(NKD):
                        xq = xq_pool.tile([128, 512], MMD, tag="xq",
                                          name="xq")
                        nc.sync.dma_start(
                            xq[:], xt[kk * 128:(kk + 1) * 128,
                                      lc * 512:(lc + 1) * 512])
                        for mh in range(2):
                            m = half * 2 + mh
                            nc.tensor.matmul(
                                ps_q[mh][:],
                                wq_t[kk][:, m * 128:(m + 1) * 128],
                                xq[:], start=(kk == 0),
                                stop=(kk == NKD - 1))
                    for mh in range(2):
                        m = half * 2 + mh
                        nc.scalar.activation(
                            qt_t[:, m * 512:(m + 1) * 512], ps_q[mh][:],
                            AF.Identity, bias=bq_t[:, m:m + 1])

                def attn_head(lc, h, qt_t, a_t):
                    # scores in 2-bank PSUM tiles; one exp per 1024 columns.
                    # AV matmuls are software-pipelined one step behind the
                    # scores so PE keeps working while ACT runs the exps.
                    acc = acc_pool.tile([128, 1024], F32R, tag="acc",
                                        name="acc")
                    ps_os = [ps_o_p.tile([128, 512], F32, tag="ps_o",
                                         name="ps_o") for _ in range(2)]

                    def av_pair(stp):
                        for et in range(2):
                            for sub in range(2):
                                stt = 2 * stp + sub
                                nc.tensor.matmul(
                                    ps_os[et][:],
                                    v_sb[stt // 4][:, (stt % 4) * EC + h * E
                                                   + et * 128:
                                                   (stt % 4) * EC + h * E
                                                   + (et + 1) * 128],
                                    a_t[stt // 4][:, (stt % 4) * 512:
                                                  (stt % 4 + 1) * 512],
                                    start=(stt == 0), stop=(stt == NST - 1))

                    for stp in range(NST // 2):
                        ps_sT = ps_sT_p.tile([128, 1024], F32, tag="ps_sT",
                                             name="ps_sT")
                        for sub in range(2):
                            stt = 2 * stp + sub
                            for et in range(2):
                                m = 2 * h + et
                                nc.tensor.matmul(
                                    ps_sT[:, sub * 512:(sub + 1) * 512],
                                    kt_sb[m][:, stt * 128:(stt + 1) * 128],
                                    qt_t[:, m * 512:(m + 1) * 512],
                                    start=(et == 0), stop=(et == 1))
                        a_ap = a_t[stp // 2][:, (stp % 2) * 1024:
                                             (stp % 2 + 1) * 1024]
                        nc.scalar.activation(a_ap, ps_sT[:], AF.Exp,
                                             scale=0.0625)
                        # accumulate denominator on DVE (in f32r so the
                        # partition-reduce matmul accepts it)
                        if stp == 0:
                            nc.vector.tensor_copy(acc[:], a_ap)
                        else:
                            nc.vector.tensor_tensor(acc[:], acc[:], a_ap,
                                                    ADD)
                        if stp >= 1:
                            av_pair(stp - 1)
                    av_pair(NST // 2 - 1)
                    # softmax denominators: fold acc halves in place, then a
                    # single ones-matrix matmul performs the partition
                    # reduction AND the broadcast (every output row = total
                    # sum); reciprocal then runs on all 128 partitions.
                    # The PSUM slot is borrowed from the (idle) qproj pool.
                    nc.vector.tensor_tensor(acc[:, 0:512], acc[:, 0:512],
                                            acc[:, 512:1024], ADD)
                    ps_b = ps_b_p.tile([128, 512], F32, tag="ps_b",
                                       name="ps_b")
                    nc.tensor.matmul(ps_b[:], ones_m[:], acc[:, 0:512],
                                     start=True, stop=True)
                    bc = bc_pool.tile([128, 512], F32, tag="bc", name="bc")
                    nc.vector.reciprocal_approx_fast(out=bc[:], in_=ps_b[:])
                    for et in range(2):
                        m = 2 * h + et
                        nc.vector.tensor_tensor(
                            o_sb[m][:, lc * 512:(lc + 1) * 512],
                            ps_os[et][:], bc[:], MUL)

                for lc in range(NLC):
                    qt_t = qt_pool.tile([128, 4 * 512], MMD, tag="qt",
                                        name="qt_t")
                    a_t = [a_pool.tile([128, 4 * 512], MMD, tag=f"a{g}",
                                       name=f"a{g}") for g in range(4)]
                    qproj_half(lc, 0, qt_t)
                    qproj_half(lc, 1, qt_t)
                    attn_head(lc, 0, qt_t, a_t)
                    attn_head(lc, 1, qt_t, a_t)

            # ---- out-projection: partial = outT.T @ WoT -> DRAM ----
            with ExitStack() as ph:
                ph.enter_context(nc.named_scope("proj"))
                wo_pool = ph.enter_context(tc.tile_pool(name="wo", bufs=4))
                pev_pool = ph.enter_context(tc.tile_pool(name="pev", bufs=8))
                psp = ph.enter_context(
                    tc.tile_pool(name="psp", bufs=4, space="PSUM"))
                for dc in range(NDC):
                    # stream the 4 Wo k-tiles for this d-chunk (1 MB)
                    wo_t = []
                    for ke in range(4):
                        w = wo_pool.tile([128, 512], MMD, tag=f"wo{ke}",
                                         name=f"wo{ke}")
                        nc.sync.dma_start(
                            w[:], wot[ke * 128:(ke + 1) * 128,
                                      dc * 512:(dc + 1) * 512])
                        wo_t.append(w)
                    for lt in range(NLT):
                        ps_p = psp.tile([128, 512], F32, tag="ps_p",
                                        name="ps_p")
                        for ke in range(4):
                            nc.tensor.matmul(
                                ps_p[:],
                                o_sb[ke][:, lt * 128:(lt + 1) * 128],
                                wo_t[ke][:, :],
                                start=(ke == 0), stop=(ke == 3))
                        ev = pev_pool.tile([128, 512], F32, tag="pev",
                                           name="pev")
                        if (lt + dc) % 2 == 0:
                            nc.vector.tensor_copy(ev[:], ps_p[:])
                        else:
                            nc.scalar.activation(ev[:], ps_p[:], AF.Copy)
                        nc.sync.dma_start(
                            out_d[lt * 128:(lt + 1) * 128,
                                  dc * 512:(dc + 1) * 512], ev[:])

    nc.compile()
    return nc


def _get_nc():
    if "nc" not in _CACHE:
        _CACHE["nc"] = _build_nc()
    return _CACHE["nc"]


def _build_in_maps(inputs):
    return _prep(**{k: inputs[k] for k in (
        "target_embedding", "source_embedding", "value_embedding",
        "Wq", "bq", "Wk", "bk", "Wv", "bv", "Wo")})


def _prep(target_embedding, source_embedding, value_embedding,
          Wq, bq, Wk, bk, Wv, bv, Wo):
    if MM_DTYPE == "bf16":
        import ml_dtypes
        mmd = ml_dtypes.bfloat16
    else:
        mmd = np.float32
    f32 = np.float32
    X = np.asarray(target_embedding, f32).reshape(BL, D)
    xt = np.ascontiguousarray(X.T)                       # [D, BL]
    st = np.ascontiguousarray(np.asarray(source_embedding, f32).T)  # [T, S]
    vt_base = np.asarray(value_embedding, f32).T         # [T, S]
    vt = np.ascontiguousarray(
        np.concatenate([vt_base, np.ones((1, S), f32)], axis=0))  # [T1, S]
    WqT = np.asarray(Wq, f32).T                          # [D, H*E]
    WkT = np.asarray(Wk, f32).T                          # [T, H*E]
    WvT = np.asarray(Wv, f32).T                          # [T, H*E]
    WoT = np.asarray(Wo, f32).T                          # [H*E, DL]
    bq = np.asarray(bq, f32)
    bk = np.asarray(bk, f32)
    bv = np.asarray(bv, f32)

    xt_c = xt.astype(mmd)
    st_c = st.astype(mmd)
    vt_c = vt.astype(mmd)
    in_maps = []
    for i in range(NCORES):
        sl = slice(i * EC, (i + 1) * EC)
        wvt_i = np.ascontiguousarray(
            np.concatenate([WvT[:, sl], bv[sl][None, :]], axis=0))  # [T1, EC]
        in_maps.append({
            "xt": xt_c,
            "st": st_c,
            "vt": vt_c,
            "wqt": np.ascontiguousarray(WqT[:, sl]).astype(mmd),
            "wkt": np.ascontiguousarray(WkT[:, sl]).astype(mmd),
            "wvt": wvt_i.astype(mmd),
            "wot": np.ascontiguousarray(WoT[sl, :]).astype(mmd),
            "bq": np.ascontiguousarray(bq[sl]),
            "bk": np.ascontiguousarray(bk[sl]),
        })
    return in_maps


def kernel(target_embedding, source_embedding, value_embedding,
           Wq, bq, Wk, bk, Wv, bv, Wo, bo):
    from concourse.bass_utils import run_bass_kernel_spmd

    in_maps = _prep(target_embedding, source_embedding, value_embedding,
                    Wq, bq, Wk, bk, Wv, bv, Wo)
    _CACHE["in_maps"] = in_maps
    nc = _get_nc()
    res = run_bass_kernel_spmd(nc, in_maps, list(range(NCORES)))

    acc = res.results[0]["out"].astype(np.float64)
    for i in range(1, NCORES):
        acc += res.results[i]["out"]
    out = (acc + np.asarray(bo, np.float64)[None, :]).astype(np.float32)
    return out.reshape(B, L, DL)


# revision 36
# speedup vs baseline: 1.1674x; 1.1674x over previous
"""Trainium2 Bass kernel for nn_MHAttentionLayer_64587718197528.

Reference computation (B=4, L=1024, D_MODEL=1024, S=2048, T=NUM_TOKENS=1000,
H=16, E=256, D_LLM=4096):
    q = (X @ Wq.T + bq)            [B*L, H*E]      X = target_embedding
    k = (SE @ Wk.T + bk)           [S, H*E]        SE = source_embedding
    v = (VE @ Wv.T + bv)           [S, H*E]        VE = value_embedding
    scores[b,h,l,s] = q . k / 16 ; A = softmax_s ; out = A @ v
    y = out @ Wo.T + bo            [B*L, D_LLM]

Sharding: tensor-parallel over heads. Core i owns heads {2i, 2i+1} (an
e-slice of 512 of the H*E dim). Each core computes its q/k/v projections,
attention for its 2 heads, and a partial out-projection
  partial_i = attn_out_i @ Wo[:, sl_i].T          [B*L, D_LLM]
The host sums the 8 partials and adds bo (linearity of the projection).

All matmul operands are float32r (full PE rate, ~1e-4 rel err). Phases:
  KV:   kT[512,2048] = Wk_i @ SE.T and v[2048,512] = VE_aug @ Wv_aug
        (bias for v folded via ones-row augmentation), SBUF-resident.
  Attn: per l-chunk of 512: q-projection (into SBUF), scoresT[s,l] in PSUM,
        exp on ACT (scale=1/16; no max subtraction -- |scaled scores| < ~8),
        softmax denominators via DVE accumulation + one ones-column matmul,
        outT[e,l] = v.T @ A.T accumulated on PE, normalized by 1/sums
        broadcast (outer-product matmul) on DVE.
  Proj: partial = outT.T @ Wo_i.T per [128,512] tile, DVE-evicted to DRAM.
"""
import numpy as np

# ---- problem constants (hardcoded per contract) ----
B, L, D = 4, 1024, 1024
S, T = 2048, 1000
H, E = 16, 256
DL = 4096
BL = B * L            # 4096 query rows
EC = 512              # e-slice per core (2 heads)
NCORES = 8
T1 = T + 1            # augmented contraction for v bias

_CACHE = {}
MM_DTYPE = "bf16"     # "f32r" (safe, ~1e-4) or "bf16" (2x PE rate, ~1e-3)


def _build_nc():
    from contextlib import ExitStack

    import concourse.tile as tile
    from concourse import bacc, mybir

    F32 = mybir.dt.float32
    F32R = mybir.dt.float32r
    MMD = mybir.dt.bfloat16 if MM_DTYPE == "bf16" else F32R
    AF = mybir.ActivationFunctionType
    MUL = mybir.AluOpType.mult
    ADD = mybir.AluOpType.add

    nc = bacc.Bacc("TRN2", target_bir_lowering=False, debug=False,
                   num_devices=NCORES)

    xt = nc.dram_tensor("xt", [D, BL], MMD, kind="ExternalInput")
    st = nc.dram_tensor("st", [T, S], MMD, kind="ExternalInput")
    vt = nc.dram_tensor("vt", [T1, S], MMD, kind="ExternalInput")
    wqt = nc.dram_tensor("wqt", [D, EC], MMD, kind="ExternalInput")
    wkt = nc.dram_tensor("wkt", [T, EC], MMD, kind="ExternalInput")
    wvt = nc.dram_tensor("wvt", [T1, EC], MMD, kind="ExternalInput")
    wot = nc.dram_tensor("wot", [EC, DL], MMD, kind="ExternalInput")
    bq_d = nc.dram_tensor("bq", [EC], F32, kind="ExternalInput")
    bk_d = nc.dram_tensor("bk", [EC], F32, kind="ExternalInput")
    out_d = nc.dram_tensor("out", [BL, DL], F32, kind="ExternalOutput")

    NKD = 8            # k-tiles for D=1024
    NKT = 8            # k-tiles for T=1000/T1=1001 (last partial)
    NLC = BL // 512    # 8 l-chunks
    NLT = BL // 128    # 32 l-tiles
    NST = S // 128     # 16 s-tiles
    NDC = DL // 512    # 8 out-proj chunks

    def kp_of(kk, total):
        return min(128, total - kk * 128)

    with tile.TileContext(nc) as tc:
        with ExitStack() as root:
            root.enter_context(
                nc.allow_low_precision(reason="fp32r matmul pipeline"))

            # ---- persistent pools ----
            consts = root.enter_context(tc.tile_pool(name="consts", bufs=1))
            kvp = root.enter_context(tc.tile_pool(name="kv", bufs=1))
            outp = root.enter_context(tc.tile_pool(name="outT", bufs=1))

            ones_f = consts.tile([128, 128], F32, name="ones_f")
            nc.vector.memset(ones_f[:], 1.0)
            ones_m = consts.tile([128, 128], F32R, name="ones_m")
            nc.vector.tensor_copy(ones_m[:], ones_f[:])
            ones_col = ones_m[:, 0:1]     # [128, 1] fp32r
            ones_row = ones_m[0:1, :]     # [1, 128] fp32r
            bqbk_t = consts.tile([128, 8], F32, name="bqbk_t")
            nc.sync.dma_start(bqbk_t[:, 0:4],
                              bq_d.ap().rearrange("(m p) -> p m", p=128))
            nc.sync.dma_start(bqbk_t[:, 4:8],
                              bk_d.ap().rearrange("(m p) -> p m", p=128))
            bq_t = bqbk_t[:, 0:4]
            bk_t = bqbk_t[:, 4:8]

            # kT: 4 e-tiles x [128, S]; v: 4 tiles [128, 4*EC] (4 s-tiles each)
            kt_sb = [kvp.tile([128, S], MMD, name=f"kt{m}", tag=f"kt{m}")
                     for m in range(4)]
            v_sb = [kvp.tile([128, 4 * EC], MMD, name=f"v{g}", tag=f"v{g}")
                    for g in range(4)]
            # outT: 4 e-tiles x [128, BL]
            o_sb = [outp.tile([128, BL], MMD, name=f"oT{m}", tag=f"oT{m}")
                    for m in range(4)]

            # wq lives in a root pool; loads are emitted after the KV
            # weight loads so they prefetch during KV compute
            wq_pool = root.enter_context(tc.tile_pool(name="wq", bufs=1))
            wq_t = []

            # ---- phase KV: kT = Wk_i @ SE.T ; v = VE_aug @ Wv_aug ----
            with ExitStack() as ph:
                ph.enter_context(nc.named_scope("kvproj"))
                wk_pool = ph.enter_context(tc.tile_pool(name="wk", bufs=1))
                wv_pool = ph.enter_context(tc.tile_pool(name="wv", bufs=1))
                sk_pool = ph.enter_context(tc.tile_pool(name="sk", bufs=4))
                sv_pool = ph.enter_context(tc.tile_pool(name="sv", bufs=4))
                psk = ph.enter_context(
                    tc.tile_pool(name="psk", bufs=1, space="PSUM"))
                psv = ph.enter_context(
                    tc.tile_pool(name="psv", bufs=1, space="PSUM"))
                wk_t, wv_t = [], []
                for kk in range(NKT):
                    kp = kp_of(kk, T)
                    w = wk_pool.tile([128, EC], MMD, name=f"wk{kk}")
                    nc.sync.dma_start(w[:kp, :], wkt[kk * 128:kk * 128 + kp, :])
                    wk_t.append(w)
                    kp1 = kp_of(kk, T1)
                    w2 = wv_pool.tile([128, EC], MMD, name=f"wv{kk}")
                    nc.sync.dma_start(w2[:kp1, :],
                                      wvt[kk * 128:kk * 128 + kp1, :])
                    wv_t.append(w2)
                for kk in range(NKD):
                    w = wq_pool.tile([128, EC], MMD, name=f"wq{kk}")
                    nc.sync.dma_start(w[:], wqt[kk * 128:(kk + 1) * 128, :])
                    wq_t.append(w)
                for sc in range(S // 512):
                    ps_k = [psk.tile([128, 512], F32, tag=f"psk{m}",
                                     name=f"psk{m}") for m in range(4)]
                    for kk in range(NKT):
                        kp = kp_of(kk, T)
                        stt = sk_pool.tile([128, 512], MMD, tag="stt",
                                           name="stt")
                        nc.sync.dma_start(
                            stt[:kp, :], st[kk * 128:kk * 128 + kp,
                                            sc * 512:(sc + 1) * 512])
                        for m in range(4):
                            nc.tensor.matmul(
                                ps_k[m][:], wk_t[kk][:kp, m * 128:(m + 1) * 128],
                                stt[:kp, :], start=(kk == 0),
                                stop=(kk == NKT - 1))
                    for m in range(4):
                        nc.scalar.activation(
                            kt_sb[m][:, sc * 512:(sc + 1) * 512], ps_k[m][:],
                            AF.Identity, bias=bk_t[:, m:m + 1])
                    # v group for the same 512-wide s range
                    g = sc
                    ps_v = [psv.tile([128, 512], F32, tag=f"psv{j}",
                                     name=f"psv{j}") for j in range(4)]
                    for kk in range(NKT):
                        kp1 = kp_of(kk, T1)
                        vtt = sv_pool.tile([128, 512], MMD, tag="vtt",
                                           name="vtt")
                        nc.sync.dma_start(
                            vtt[:kp1, :], vt[kk * 128:kk * 128 + kp1,
                                             g * 512:(g + 1) * 512])
                        for j in range(4):
                            nc.tensor.matmul(
                                ps_v[j][:], vtt[:kp1, j * 128:(j + 1) * 128],
                                wv_t[kk][:kp1, :], start=(kk == 0),
                                stop=(kk == NKT - 1))
                    for j in range(4):
                        nc.scalar.activation(
                            v_sb[g][:, j * EC:(j + 1) * EC], ps_v[j][:],
                            AF.Copy)

            # ---- fused attention phase (q-projection + attention per lc) ----
            with ExitStack() as ph:
                ph.enter_context(nc.named_scope("attn"))
                xq_pool = ph.enter_context(tc.tile_pool(name="xq", bufs=4))
                qt_pool = ph.enter_context(tc.tile_pool(name="qtp", bufs=2))
                a_pool = ph.enter_context(tc.tile_pool(name="ap", bufs=2))
                acc_pool = ph.enter_context(tc.tile_pool(name="accp", bufs=1))
                bc_pool = ph.enter_context(tc.tile_pool(name="bcp", bufs=2))
                psq_p = ph.enter_context(
                    tc.tile_pool(name="psq", bufs=1, space="PSUM"))
                ps_sT_p = ph.enter_context(
                    tc.tile_pool(name="ps_sT", bufs=1, space="PSUM"))
                ps_b_p = ph.enter_context(
                    tc.tile_pool(name="ps_b", bufs=2, space="PSUM"))
                ps_o_p = ph.enter_context(
                    tc.tile_pool(name="ps_o", bufs=2, space="PSUM"))

                def qproj_half(lc, half, qt_t):
                    ps_q = [psq_p.tile([128, 512], F32, tag=f"psq{mh}",
                                       name=f"psq{mh}") for mh in range(2)]
                    for kk in range(NKD):
                        xq = xq_pool.tile([128, 512], MMD, tag="xq",
                                          name="xq")
                        nc.sync.dma_start(
                            xq[:], xt[kk * 128:(kk + 1) * 128,
                                      lc * 512:(lc + 1) * 512])
                        for mh in range(2):
                            m = half * 2 + mh
                            nc.tensor.matmul(
                                ps_q[mh][:],
                                wq_t[kk][:, m * 128:(m + 1) * 128],
                                xq[:], start=(kk == 0),
                                stop=(kk == NKD - 1))
                    for mh in range(2):
                        m = half * 2 + mh
                        nc.scalar.activation(
                            qt_t[:, m * 512:(m + 1) * 512], ps_q[mh][:],
                            AF.Identity, bias=bq_t[:, m:m + 1])

                def attn_head(lc, h, qt_t, a_t):
                    # scores in 2-bank PSUM tiles; one exp per 1024 columns.
                    # AV matmuls are software-pipelined one step behind the
                    # scores so PE keeps working while ACT runs the exps.
                    acc = acc_pool.tile([128, 1024], F32R, tag="acc",
                                        name="acc")
                    ps_os = [ps_o_p.tile([128, 512], F32, tag="ps_o",
                                         name="ps_o") for _ in range(2)]

                    def av_pair(stp):
                        for et in range(2):
                            for sub in range(2):
                                stt = 2 * stp + sub
                                nc.tensor.matmul(
                                    ps_os[et][:],
                                    v_sb[stt // 4][:, (stt % 4) * EC + h * E
                                                   + et * 128:
                                                   (stt % 4) * EC + h * E
                                                   + (et + 1) * 128],
                                    a_t[stt // 4][:, (stt % 4) * 512:
                                                  (stt % 4 + 1) * 512],
                                    start=(stt == 0), stop=(stt == NST - 1))

                    for stp in range(NST // 2):
                        ps_sT = ps_sT_p.tile([128, 1024], F32, tag="ps_sT",
                                             name="ps_sT")
                        for sub in range(2):
                            stt = 2 * stp + sub
                            for et in range(2):
                                m = 2 * h + et
                                nc.tensor.matmul(
                                    ps_sT[:, sub * 512:(sub + 1) * 512],
                                    kt_sb[m][:, stt * 128:(stt + 1) * 128],
                                    qt_t[:, m * 512:(m + 1) * 512],
                                    start=(et == 0), stop=(et == 1))
                        a_ap = a_t[stp // 2][:, (stp % 2) * 1024:
                                             (stp % 2 + 1) * 1024]
                        nc.scalar.activation(a_ap, ps_sT[:], AF.Exp,
                                             scale=0.0625)
                        # accumulate denominator on DVE (in f32r so the
                        # partition-reduce matmul accepts it)
                        if stp == 0:
                            nc.vector.tensor_copy(acc[:], a_ap)
                        else:
                            nc.vector.tensor_tensor(acc[:], acc[:], a_ap,
                                                    ADD)
                        if stp >= 1:
                            av_pair(stp - 1)
                    av_pair(NST // 2 - 1)
                    # softmax denominators: fold acc halves in place, then a
                    # single ones-matrix matmul performs the partition
                    # reduction AND the broadcast (every output row = total
                    # sum); reciprocal then runs on all 128 partitions.
                    # The PSUM slot is borrowed from the (idle) qproj pool.
                    nc.vector.tensor_tensor(acc[:, 0:512], acc[:, 0:512],
                                            acc[:, 512:1024], ADD)
                    ps_b = ps_b_p.tile([128, 512], F32, tag="ps_b",
                                       name="ps_b")
                    nc.tensor.matmul(ps_b[:], ones_m[:], acc[:, 0:512],
                                     start=True, stop=True)
                    bc = bc_pool.tile([128, 512], F32, tag="bc", name="bc")
                    nc.vector.reciprocal_approx_fast(out=bc[:], in_=ps_b[:])
                    for et in range(2):
                        m = 2 * h + et
                        nc.vector.tensor_tensor(
                            o_sb[m][:, lc * 512:(lc + 1) * 512],
                            ps_os[et][:], bc[:], MUL)

                for lc in range(NLC):
                    qt_t = qt_pool.tile([128, 4 * 512], MMD, tag="qt",
                                        name="qt_t")
                    a_t = [a_pool.tile([128, 4 * 512], MMD, tag=f"a{g}",
                                       name=f"a{g}") for g in range(4)]
                    qproj_half(lc, 0, qt_t)
                    qproj_half(lc, 1, qt_t)
                    attn_head(lc, 0, qt_t, a_t)
                    attn_head(lc, 1, qt_t, a_t)

            # ---- out-projection: partial = outT.T @ WoT -> DRAM ----
            with ExitStack() as ph:
                ph.enter_context(nc.named_scope("proj"))
                wo_pool = ph.enter_context(tc.tile_pool(name="wo", bufs=4))
                pev_pool = ph.enter_context(tc.tile_pool(name="pev", bufs=8))
                psp = ph.enter_context(
                    tc.tile_pool(name="psp", bufs=4, space="PSUM"))
                for dc in range(NDC):
                    # stream the 4 Wo k-tiles for this d-chunk (1 MB)
                    wo_t = []
                    for ke in range(4):
                        w = wo_pool.tile([128, 512], MMD, tag=f"wo{ke}",
                                         name=f"wo{ke}")
                        nc.sync.dma_start(
                            w[:], wot[ke * 128:(ke + 1) * 128,
                                      dc * 512:(dc + 1) * 512])
                        wo_t.append(w)
                    for lt in range(NLT):
                        ps_p = psp.tile([128, 512], F32, tag="ps_p",
                                        name="ps_p")
                        for ke in range(4):
                            nc.tensor.matmul(
                                ps_p[:],
                                o_sb[ke][:, lt * 128:(lt + 1) * 128],
                                wo_t[ke][:, :],
                                start=(ke == 0), stop=(ke == 3))
                        ev = pev_pool.tile([128, 512], F32, tag="pev",
                                           name="pev")
                        nc.vector.tensor_copy(ev[:], ps_p[:])
                        nc.sync.dma_start(
                            out_d[lt * 128:(lt + 1) * 128,
                                  dc * 512:(dc + 1) * 512], ev[:])

    nc.compile()
    return nc


def _get_nc():
    if "nc" not in _CACHE:
        _CACHE["nc"] = _build_nc()
    return _CACHE["nc"]


def _build_in_maps(inputs):
    return _prep(**{k: inputs[k] for k in (
        "target_embedding", "source_embedding", "value_embedding",
        "Wq", "bq", "Wk", "bk", "Wv", "bv", "Wo")})


def _prep(target_embedding, source_embedding, value_embedding,
          Wq, bq, Wk, bk, Wv, bv, Wo):
    if MM_DTYPE == "bf16":
        import ml_dtypes
        mmd = ml_dtypes.bfloat16
    else:
        mmd = np.float32
    f32 = np.float32
    X = np.asarray(target_embedding, f32).reshape(BL, D)
    xt = np.ascontiguousarray(X.T)                       # [D, BL]
    st = np.ascontiguousarray(np.asarray(source_embedding, f32).T)  # [T, S]
    vt_base = np.asarray(value_embedding, f32).T         # [T, S]
    vt = np.ascontiguousarray(
        np.concatenate([vt_base, np.ones((1, S), f32)], axis=0))  # [T1, S]
    WqT = np.asarray(Wq, f32).T                          # [D, H*E]
    WkT = np.asarray(Wk, f32).T                          # [T, H*E]
    WvT = np.asarray(Wv, f32).T                          # [T, H*E]
    WoT = np.asarray(Wo, f32).T                          # [H*E, DL]
    bq = np.asarray(bq, f32)
    bk = np.asarray(bk, f32)
    bv = np.asarray(bv, f32)

    xt_c = xt.astype(mmd)
    st_c = st.astype(mmd)
    vt_c = vt.astype(mmd)
    in_maps = []
    for i in range(NCORES):
        sl = slice(i * EC, (i + 1) * EC)
        wvt_i = np.ascontiguousarray(
            np.concatenate([WvT[:, sl], bv[sl][None, :]], axis=0))  # [T1, EC]
        in_maps.append({
            "xt": xt_c,
            "st": st_c,
            "vt": vt_c,
            "wqt": np.ascontiguousarray(WqT[:, sl]).astype(mmd),
            "wkt": np.ascontiguousarray(WkT[:, sl]).astype(mmd),
            "wvt": wvt_i.astype(mmd),
            "wot": np.ascontiguousarray(WoT[sl, :]).astype(mmd),
            "bq": np.ascontiguousarray(bq[sl]),
            "bk": np.ascontiguousarray(bk[sl]),
        })
    return in_maps


def kernel(target_embedding, source_embedding, value_embedding,
           Wq, bq, Wk, bk, Wv, bv, Wo, bo):
    from concourse.bass_utils import run_bass_kernel_spmd

    in_maps = _prep(target_embedding, source_embedding, value_embedding,
                    Wq, bq, Wk, bk, Wv, bv, Wo)
    _CACHE["in_maps"] = in_maps
    nc = _get_nc()
    res = run_bass_kernel_spmd(nc, in_maps, list(range(NCORES)))

    acc = res.results[0]["out"].astype(np.float64)
    for i in range(1, NCORES):
        acc += res.results[i]["out"]
    out = (acc + np.asarray(bo, np.float64)[None, :]).astype(np.float32)
    return out.reshape(B, L, DL)
